# revision 7
# baseline (speedup 1.0000x reference)
"""Routed-MoE kernel for Trainium2 (8 NeuronCores).

Reference computes all-experts MLP logits for every token and then gathers
the expert chosen by `domain`.  Only the selected expert's output is needed,
so this kernel routes on the host (argsort by expert) and runs one expert
per NeuronCore over its (capacity-padded) token group:

    core e:  out = softmax(relu(Xg[e] @ W1[e] + b1[e]) @ W2[e] + b2[e])

Layouts are chosen so no on-device transposes are needed:
  - L1 computes H^T [F2, tok] with lhsT = W1 (native [F1,F2] layout) and
    rhs = Xg^T (host-transposed gather).
  - L2 computes logits [tok, C] with lhsT = H^T tiles and rhs = W2 (native).
Matmul inputs are bitcast to float32r (FP22 single-pass mode, ~4x the
throughput of true 4-pass FP32; PSUM accumulation stays FP32).
"""

import os
import numpy as np
from contextlib import ExitStack

import concourse.bass as bass
import concourse.bacc as bacc
import concourse.tile as tile
from concourse import mybir
from concourse.bass_utils import run_bass_kernel_spmd

B, E, F1, F2, C = 8192, 8, 1024, 2048, 100
N_CORES = 8
CAP = 1280  # per-expert token capacity (binomial(8192,1/8): mean 1024, sd ~30)
P = 128
K1 = F1 // P   # 8  K-tiles for layer 1
M1 = F2 // P   # 16 M-tiles for layer 1 (= K-tiles for layer 2)
NT = CAP // P  # 10 token tiles
SLICES = [(0, 512), (512, 512), (1024, 256)]  # token slices for layer 1

F32 = mybir.dt.float32
F32R = mybir.dt.float32r

_CACHED_NC = None


def _build_nc():
    nc = bacc.Bacc("TRN2", target_bir_lowering=False, debug=False,
                   num_devices=N_CORES)
    xT_d = nc.dram_tensor("xT", [F1, CAP], F32R, kind="ExternalInput").ap()
    w1_d = nc.dram_tensor("w1", [F1, F2], F32R, kind="ExternalInput").ap()
    b1_d = nc.dram_tensor("b1r", [P, M1], F32, kind="ExternalInput").ap()
    w2_d = nc.dram_tensor("w2", [F2, C], F32R, kind="ExternalInput").ap()
    b2_d = nc.dram_tensor("b2b", [P, C], F32, kind="ExternalInput").ap()
    out_d = nc.dram_tensor("out", [CAP, C], F32, kind="ExternalOutput").ap()

    with tile.TileContext(nc) as tc, ExitStack() as ctx:
        const = ctx.enter_context(tc.tile_pool(name="const", bufs=1))
        hpool = ctx.enter_context(tc.tile_pool(name="h", bufs=2))
        ps1 = ctx.enter_context(tc.tile_pool(name="ps1", bufs=4, space="PSUM"))
        ps2 = ctx.enter_context(tc.tile_pool(name="ps2", bufs=4, space="PSUM"))
        spool = ctx.enter_context(tc.tile_pool(name="stats", bufs=8))
        opool = ctx.enter_context(tc.tile_pool(name="out", bufs=8))

        # Resident weights/activations.  w1sb[p, k*F2+j] = W1[k*128+p, j]
        w1r = const.tile([P, K1 * F2], F32R)
        for k in range(K1):
            nc.sync.dma_start(w1r[:, k * F2:(k + 1) * F2],
                              w1_d[k * P:(k + 1) * P, :])
        # xsb[p, k*CAP+t] = xT[k*128+p, t]
        xr = const.tile([P, K1 * CAP], F32R)
        for k in range(K1):
            nc.sync.dma_start(xr[:, k * CAP:(k + 1) * CAP],
                              xT_d[k * P:(k + 1) * P, :])
        # w2sb[p, m*C+c] = W2[m*128+p, c]
        w2r = const.tile([P, M1 * C], F32R)
        for m in range(M1):
            nc.sync.dma_start(w2r[:, m * C:(m + 1) * C],
                              w2_d[m * P:(m + 1) * P, :])
        b1sb = const.tile([P, M1], F32)
        nc.sync.dma_start(b1sb[:], b1_d[:])
        b2sb = const.tile([P, C], F32)
        nc.sync.dma_start(b2sb[:], b2_d[:])

        for n0, S in SLICES:
            # Layer 1: H^T slice [F2, S] = W1^T(lhsT) contracted with Xg^T
            hr = hpool.tile([P, M1 * 512], F32R, tag="h")
            for m in range(M1):
                ps = ps1.tile([P, 512], F32)
                for k in range(K1):
                    nc.tensor.matmul(
                        ps[:, :S],
                        w1r[:, k * F2 + m * P: k * F2 + (m + 1) * P],
                        xr[:, k * CAP + n0: k * CAP + n0 + S],
                        start=(k == 0), stop=(k == K1 - 1),
                    )
                # h[:, m*512 : m*512+S] = relu(psum + b1[m-chunk])
                nc.scalar.activation(
                    hr[:, m * 512: m * 512 + S], ps[:, :S],
                    mybir.ActivationFunctionType.Relu,
                    bias=b1sb[:, m:m + 1],
                )

            # Layer 2 + softmax per 128-token tile
            for j in range(S // P):
                ps_l = ps2.tile([P, C], F32)
                for m in range(M1):
                    nc.tensor.matmul(
                        ps_l[:],
                        hr[:, m * 512 + j * P: m * 512 + (j + 1) * P],
                        w2r[:, m * C:(m + 1) * C],
                        start=(m == 0), stop=(m == M1 - 1),
                    )
                logits = opool.tile([P, C], F32, tag="logits")
                nc.vector.tensor_add(logits[:], ps_l[:], b2sb[:])
                nmx = spool.tile([P, 1], F32, tag="nmx")
                nc.vector.tensor_reduce(nmx[:], logits[:],
                                        axis=mybir.AxisListType.X,
                                        op=mybir.AluOpType.max, negate=True)
                ex = opool.tile([P, C], F32, tag="ex")
                sm = spool.tile([P, 1], F32, tag="sm")
                nc.scalar.activation(ex[:], logits[:],
                                     mybir.ActivationFunctionType.Exp,
                                     bias=nmx[:], accum_out=sm[:])
                rc = spool.tile([P, 1], F32, tag="rc")
                nc.vector.reciprocal(rc[:], sm[:])
                prob = opool.tile([P, C], F32, tag="prob")
                nc.vector.tensor_scalar_mul(prob[:], ex[:], rc[:])
                row0 = n0 + j * P
                nc.sync.dma_start(out_d[row0:row0 + P, :], prob[:])

    nc.compile()
    return nc


def _get_nc():
    global _CACHED_NC
    if _CACHED_NC is None:
        _CACHED_NC = _build_nc()
    return _CACHED_NC


def _np_mlp_rows(x_rows, e, W1, b1, W2, b2):
    """Host fallback (exact fp32 semantics) for capacity-overflow tokens."""
    h = np.maximum(x_rows.astype(np.float32) @ W1[e] + b1[e], 0.0)
    logits = h @ W2[e] + b2[e]
    logits -= logits.max(axis=-1, keepdims=True)
    p = np.exp(logits)
    return (p / p.sum(axis=-1, keepdims=True)).astype(np.float32)


def kernel(domain, x, W1, b1, W2, b2):
    domain = np.asarray(domain).astype(np.int64)
    x = np.ascontiguousarray(np.asarray(x, dtype=np.float32))
    W1 = np.asarray(W1, dtype=np.float32)
    b1 = np.asarray(b1, dtype=np.float32)
    W2 = np.asarray(W2, dtype=np.float32)
    b2 = np.asarray(b2, dtype=np.float32)

    order = np.argsort(domain, kind="stable")
    counts = np.bincount(domain, minlength=E).astype(np.int64)
    starts = np.concatenate([[0], np.cumsum(counts)[:-1]])

    xT = x.T  # [F1, B] view
    in_maps = []
    kept_idx = []
    for e in range(E):
        n_e = int(min(counts[e], CAP))
        idx = order[starts[e]: starts[e] + n_e]
        kept_idx.append(idx)
        xg = np.zeros((F1, CAP), dtype=np.float32)
        xg[:, :n_e] = xT[:, idx]
        in_maps.append({
            "xT": xg,
            "w1": np.ascontiguousarray(W1[e]),
            "b1r": np.ascontiguousarray(b1[e].reshape(M1, P).T),
            "w2": np.ascontiguousarray(W2[e]),
            "b2b": np.ascontiguousarray(np.broadcast_to(b2[e], (P, C))),
        })

    nc = _get_nc()
    res = run_bass_kernel_spmd(nc, in_maps, list(range(N_CORES)))

    out = np.empty((B, C), dtype=np.float32)
    for e in range(E):
        idx = kept_idx[e]
        out[idx] = res.results[e]["out"][: len(idx)]
        if counts[e] > CAP:  # astronomically unlikely; exact host fallback
            ov = order[starts[e] + CAP: starts[e] + counts[e]]
            out[ov] = _np_mlp_rows(x[ov], e, W1, b1, W2, b2)

    return out


# revision 8
# speedup vs baseline: 1.5140x; 1.5140x over previous
"""Routed-MoE kernel for Trainium2 (8 NeuronCores).

The reference computes all-experts MLP logits for every token and then
gathers the expert chosen by `domain`.  Only the selected expert's output is
needed, so this kernel routes on the host (argsort by expert) and runs one
expert per NeuronCore over its (capacity-padded) token group:

    core e:  out = softmax(relu(Xg[e] @ W1[e] + b1[e]) @ W2[e] + b2[e])

Key layout/performance choices:
  - No on-device transposes on the main path: L1 computes H^T [F2, tok]
    with lhsT = W1 tiles (host pre-swizzled per-m so one DMA unlocks one
    m-group) and rhs = Xg^T (host-transposed gather, streamed per
    (k, token-slice) so the PE can start after ~2 MB of DMA).
  - All matmuls run in float32r (FP22 single-pass mode, 4x true-FP32
    throughput; PSUM accumulation stays FP32).  Token slices are 384 wide
    so every matmul has output free dim >= 256 (full f32r rate).
  - L2 is computed as logits^T [C, tok] (free dim = tokens >= 256), then
    PE-transposed per 128-token tile for the free-axis softmax.
  - Token slices (0,1) are interleaved inside the m-loop so PE never
    outpaces the W1 DMA stream; slice 2 runs after weights are resident.
"""

import numpy as np
from contextlib import ExitStack

import concourse.bass as bass
import concourse.bacc as bacc
import concourse.tile as tile
from concourse import mybir
from concourse.bass_utils import run_bass_kernel_spmd
from concourse.masks import make_identity

B, E, F1, F2, C = 8192, 8, 1024, 2048, 100
N_CORES = 8
P = 128
SW = 384            # token-slice width (>=256 keeps f32r at full rate)
NSLICE = 3
CAP = SW * NSLICE   # 1152 per-expert capacity (binomial mean 1024, sd ~30)
K1 = F1 // P        # 8  K-tiles for layer 1
M1 = F2 // P        # 16 M-tiles for layer 1 (= K-tiles for layer 2)

F32 = mybir.dt.float32
F32R = mybir.dt.float32r

_CACHED_NC = None


def _build_nc():
    nc = bacc.Bacc("TRN2", target_bir_lowering=False, debug=False,
                   num_devices=N_CORES)
    xT_d = nc.dram_tensor("xT", [F1, CAP], F32R, kind="ExternalInput").ap()
    # w1s host-swizzled: w1s[m*128 + p, k*128 + j] = W1[k*128 + p, m*128 + j]
    w1_d = nc.dram_tensor("w1s", [F2, F1], F32R, kind="ExternalInput").ap()
    b1_d = nc.dram_tensor("b1r", [P, M1], F32, kind="ExternalInput").ap()
    w2_d = nc.dram_tensor("w2", [F2, C], F32R, kind="ExternalInput").ap()
    b2_d = nc.dram_tensor("b2b", [P, C], F32, kind="ExternalInput").ap()
    out_d = nc.dram_tensor("out", [CAP, C], F32, kind="ExternalOutput").ap()

    with tile.TileContext(nc) as tc, ExitStack() as ctx:
        const = ctx.enter_context(tc.tile_pool(name="const", bufs=1))
        hpool = ctx.enter_context(tc.tile_pool(name="h", bufs=1))
        ps1 = ctx.enter_context(tc.tile_pool(name="ps1", bufs=3, space="PSUM"))
        ps2 = ctx.enter_context(tc.tile_pool(name="ps2", bufs=2, space="PSUM"))
        ps3 = ctx.enter_context(tc.tile_pool(name="ps3", bufs=2, space="PSUM"))
        lpool = ctx.enter_context(tc.tile_pool(name="l2sb", bufs=2))
        spool = ctx.enter_context(tc.tile_pool(name="stats", bufs=8))
        opool = ctx.enter_context(tc.tile_pool(name="out", bufs=8))

        ident = const.tile([P, P], F32, tag="ident")
        make_identity(nc, ident[:])
        b1sb = const.tile([P, M1], F32, tag="b1")
        nc.sync.dma_start(b1sb[:], b1_d[:])

        # x streamed per (k, slice); slices 0/1 first so compute starts early
        xt = {}
        for s in (0, 1):
            for k in range(K1):
                t = const.tile([P, SW], F32R, tag=f"x_{k}_{s}", name=f"x_{k}_{s}")
                nc.sync.dma_start(
                    t[:], xT_d[k * P:(k + 1) * P, s * SW:(s + 1) * SW])
                xt[k, s] = t
        # W1: one DMA per m-group (host-swizzled rows are contiguous)
        w1t = []
        for m in range(M1):
            t = const.tile([P, K1 * P], F32R, tag=f"w1_{m}", name=f"w1_{m}")
            nc.sync.dma_start(t[:], w1_d[m * P:(m + 1) * P, :])
            w1t.append(t)
        for k in range(K1):
            s = 2
            t = const.tile([P, SW], F32R, tag=f"x_{k}_{s}", name=f"x_{k}_{s}")
            nc.sync.dma_start(
                t[:], xT_d[k * P:(k + 1) * P, s * SW:(s + 1) * SW])
            xt[k, s] = t
        w2t = []
        for m in range(M1):
            t = const.tile([P, C], F32R, tag=f"w2_{m}", name=f"w2_{m}")
            nc.sync.dma_start(t[:], w2_d[m * P:(m + 1) * P, :])
            w2t.append(t)
        b2sb = const.tile([P, C], F32, tag="b2")
        nc.sync.dma_start(b2sb[:], b2_d[:])

        # H^T for the whole batch: h[p, m*CAP + s*SW + t]
        h = hpool.tile([P, M1 * CAP], F32R, tag="h")

        def l1_group(m, s):
            ps = ps1.tile([P, SW], F32, tag="ps1", name=f"ps1_{m}_{s}")
            for k in range(K1):
                nc.tensor.matmul(
                    ps[:], w1t[m][:, k * P:(k + 1) * P], xt[k, s][:],
                    start=(k == 0), stop=(k == K1 - 1))
            nc.scalar.activation(
                h[:, m * CAP + s * SW: m * CAP + (s + 1) * SW], ps[:],
                mybir.ActivationFunctionType.Relu, bias=b1sb[:, m:m + 1])

        def l2_softmax(s):
            # logits^T [C, SW] at full f32r rate, then per-128-token
            # PE-transpose + free-axis softmax.
            psl = ps2.tile([C, SW], F32, tag="psl", name=f"psl_{s}")
            for m in range(M1):
                nc.tensor.matmul(
                    psl[:], w2t[m][:], h[:, m * CAP + s * SW: m * CAP + (s + 1) * SW],
                    start=(m == 0), stop=(m == M1 - 1))
            l2sb = lpool.tile([C, SW], F32, tag="l2sb", name=f"l2sb_{s}")
            nc.scalar.copy(l2sb[:], psl[:])
            for j in range(SW // P):
                pst = ps3.tile([P, C], F32, tag="pst", name=f"pst_{s}_{j}")
                nc.tensor.transpose(pst[:], l2sb[:, j * P:(j + 1) * P],
                                    ident[:C, :C])
                logits = opool.tile([P, C], F32, tag="logits")
                nc.vector.tensor_add(logits[:], pst[:], b2sb[:])
                nmx = spool.tile([P, 1], F32, tag="nmx")
                nc.vector.tensor_reduce(nmx[:], logits[:],
                                        axis=mybir.AxisListType.X,
                                        op=mybir.AluOpType.max, negate=True)
                ex = opool.tile([P, C], F32, tag="ex")
                sm = spool.tile([P, 1], F32, tag="sm")
                nc.scalar.activation(ex[:], logits[:],
                                     mybir.ActivationFunctionType.Exp,
                                     bias=nmx[:], accum_out=sm[:])
                rc = spool.tile([P, 1], F32, tag="rc")
                nc.vector.reciprocal(rc[:], sm[:])
                prob = opool.tile([P, C], F32, tag="prob")
                nc.vector.tensor_scalar_mul(prob[:], ex[:], rc[:])
                row0 = s * SW + j * P
                nc.sync.dma_start(out_d[row0:row0 + P, :], prob[:])

        # Slices 0/1 interleaved in the m-loop: PE consumes one w1 m-tile
        # per ~2.5us while its DMA takes ~1.9us -> no weight-stream stall.
        for m in range(M1):
            l1_group(m, 0)
            l1_group(m, 1)
        l2_softmax(0)
        l2_softmax(1)
        for m in range(M1):
            l1_group(m, 2)
        l2_softmax(2)

    nc.compile()
    return nc


def _get_nc():
    global _CACHED_NC
    if _CACHED_NC is None:
        _CACHED_NC = _build_nc()
    return _CACHED_NC


def _np_mlp_rows(x_rows, e, W1, b1, W2, b2):
    """Host fallback (exact fp32 semantics) for capacity-overflow tokens."""
    h = np.maximum(x_rows.astype(np.float32) @ W1[e] + b1[e], 0.0)
    logits = h @ W2[e] + b2[e]
    logits -= logits.max(axis=-1, keepdims=True)
    p = np.exp(logits)
    return (p / p.sum(axis=-1, keepdims=True)).astype(np.float32)


def kernel(domain, x, W1, b1, W2, b2):
    domain = np.asarray(domain).astype(np.int64)
    x = np.ascontiguousarray(np.asarray(x, dtype=np.float32))
    W1 = np.asarray(W1, dtype=np.float32)
    b1 = np.asarray(b1, dtype=np.float32)
    W2 = np.asarray(W2, dtype=np.float32)
    b2 = np.asarray(b2, dtype=np.float32)

    order = np.argsort(domain, kind="stable")
    counts = np.bincount(domain, minlength=E).astype(np.int64)
    starts = np.concatenate([[0], np.cumsum(counts)[:-1]])

    xT = x.T  # [F1, B] view
    in_maps = []
    kept_idx = []
    for e in range(E):
        n_e = int(min(counts[e], CAP))
        idx = order[starts[e]: starts[e] + n_e]
        kept_idx.append(idx)
        xg = np.zeros((F1, CAP), dtype=np.float32)
        xg[:, :n_e] = xT[:, idx]
        # w1s[m*128+p, k*128+j] = W1[e][k*128+p, m*128+j]
        w1s = np.ascontiguousarray(
            W1[e].reshape(K1, P, M1, P).transpose(2, 1, 0, 3).reshape(F2, F1))
        in_maps.append({
            "xT": xg,
            "w1s": w1s,
            "b1r": np.ascontiguousarray(b1[e].reshape(M1, P).T),
            "w2": np.ascontiguousarray(W2[e]),
            "b2b": np.ascontiguousarray(np.broadcast_to(b2[e], (P, C))),
        })

    nc = _get_nc()
    res = run_bass_kernel_spmd(nc, in_maps, list(range(N_CORES)))

    out = np.empty((B, C), dtype=np.float32)
    for e in range(E):
        idx = kept_idx[e]
        out[idx] = res.results[e]["out"][: len(idx)]
        if counts[e] > CAP:  # astronomically unlikely; exact host fallback
            ov = order[starts[e] + CAP: starts[e] + counts[e]]
            out[ov] = _np_mlp_rows(x[ov], e, W1, b1, W2, b2)

    return out
